# revision 31
# baseline (speedup 1.0000x reference)
"""Trainium2 Bass kernel for a group-conv / orbit-shared message-passing layer.

Math: out[b, i, o] = sum_{j,c} weight[o, c, pair_orbit[i, j]] * x[b, j, c] + bias[o]

Strategy (pure data parallel over 8 NeuronCores):
  * Host gathers the orbit-shared weight into per-output-position matrices
    W_i[(j,c), o] (24 matrices of 1536x64), regrouped as moving operands
    Wmov[k, g][kc, (di,o)] of [128, 512] covering 8 output positions each.
  * Host transposes x to x^T[(j,c), b] so the contraction dim (j,c)=1536 sits
    on SBUF partitions; each core takes B/8 = 4096 batch columns.
  * Per 128-batch tile: stationary = x^T k-tile [kc=128, b=128] (one weight
    load per 3 matmuls with walrus ldw-opt dedupe), moving = Wmov[k, g]
    [kc=128, 512]; 12 k-tiles accumulate into 3 PSUM banks:
        psum_g[b, (di,o)] += xT[kc, b].T @ Wmov[k,g][kc, (di,o)]
    The PSUM free axis (di,o) is already the natural out[b, i, o] layout, so
    stores go straight to a (4096, 24*64) DRAM tensor. No host-side output
    transpose.
  * DMA queues are split by stream: x loads on sync, weight chunks on scalar,
    output stores on gpsimd -- so the big weight burst at startup does not
    queue behind x tiles and the output never backs up the input stream.
"""

import sys

for _p in ("/opt/trn_rl_repo",):
    if _p not in sys.path:
        sys.path.insert(0, _p)

import numpy as np
import ml_dtypes

import concourse.bacc as bacc
import concourse.mybir as mybir
from concourse import tile
from concourse.bass_utils import run_bass_kernel_spmd

B, P, C_IN, C_OUT, N_ORB = 32768, 24, 64, 64, 24
N_CORES = 8
BL = B // N_CORES            # 4096 batch per core
JC = P * C_IN                # 1536 contraction size
KT = JC // 128               # 12 K-tiles
NG = 3                       # output groups of 8 positions (8*64 = 512 free)
NBT = BL // 128              # 32 batch tiles per core

# "bf16" | "f32r" | "f32"
COMPUTE_DTYPE = "f32r"
# Ship weights over the wire as bf16 (half the startup DMA) and cast them to
# the compute dtype on-device with the Vector engine, which is idle during
# startup. Mixed-dtype matmuls are rejected by walrus, so the cast is needed.
W_BF16_WIRE = True
# Let walrus dedupe back-to-back LDWEIGHTS of the same stationary operand.
# Measured: dedupe is a net loss here (~+6ns/MM steady-state: the per-k-tile
# LDWEIGHTS->MATMUL serialization outweighs the removed instructions).
LDW_OPT = False
# Dummy matmuls issued while the first DMAs are in flight, so the HAM clock
# gate reaches 8/8 before the first real matmul.
WARMUP_MMS = 10

_CACHE = {}


def _patch_ldw_opt():
    import concourse.bass_utils as bu

    orig = bu.run_command
    if getattr(orig, "_ldw_patched", False):
        return

    def wrapper(argv, **kwargs):
        if LDW_OPT and "--enable-ldw-opt=false" in argv:
            argv = ["--enable-ldw-opt=true" if a == "--enable-ldw-opt=false" else a
                    for a in argv]
        return orig(argv, **kwargs)

    wrapper._ldw_patched = True
    bu.run_command = wrapper


def _dt(dt_tag):
    if dt_tag == "bf16":
        return mybir.dt.bfloat16
    if dt_tag == "f32r":
        return mybir.dt.float32r
    return mybir.dt.float32


def _build(dt_tag):
    _patch_ldw_opt()
    DT = _dt(dt_tag)
    wire_bf16 = W_BF16_WIRE and dt_tag != "bf16"
    DTW = mybir.dt.bfloat16 if wire_bf16 else DT

    nc = bacc.Bacc(None, target_bir_lowering=False, debug=False)
    # x pre-packed on host so each batch tile is one contiguous [128, 1536]
    # block (per-partition 6KB runs -> large DMA descriptors, not 512B shreds)
    xt = nc.dram_tensor("xt", [NBT, 128, KT * 128], DT, kind="ExternalInput")
    w = nc.dram_tensor("w", [128, KT * NG * 512], DTW, kind="ExternalInput")
    # row-major output: batch-tile rows are contiguous 768KB stores
    out_l = nc.dram_tensor("out_l", [BL, P * C_OUT], mybir.dt.float32,
                           kind="ExternalOutput")

    with tile.TileContext(nc) as tc:
        with (
            tc.tile_pool(name="wpool", bufs=1) as wpool,
            tc.tile_pool(name="wsta", bufs=1) as wstage,
            tc.tile_pool(name="xpool", bufs=3) as xpool,
            tc.tile_pool(name="opool", bufs=4) as opool,
            tc.tile_pool(name="pspool", bufs=2, space="PSUM") as pspool,
        ):
            def _vcast(out, in_):
                nc.vector.tensor_copy(out, in_)

            def _scast(out, in_):
                nc.scalar.copy(out, in_)

            cast_engs = [_vcast, _scast, _vcast]

            def w_chunk(k, eng):
                sl = slice(k * NG * 512, (k + 1) * NG * 512)
                if not wire_bf16:
                    wc = wpool.tile([128, NG * 512], DT, tag=f"w{k}",
                                    name=f"wc{k}")
                    eng.dma_start(wc[:], w.ap()[:, sl])
                    return wc
                # unique staging tile per chunk: chunk DMAs never wait on a
                # cast to release a reused buffer
                ws = wstage.tile([128, NG * 512], mybir.dt.bfloat16,
                                 tag=f"ws{k}", name=f"ws{k}")
                eng.dma_start(ws[:], w.ap()[:, sl])
                wc = wpool.tile([128, NG * 512], DT, tag=f"w{k}",
                                name=f"wc{k}")
                # per-group casts spread over the idle compute engines
                for g in range(NG):
                    cast_engs[g](
                        wc[:, g * 512:(g + 1) * 512],
                        ws[:, g * 512:(g + 1) * 512],
                    )
                return wc

            # HAM warmup: the PE sits idle for ~5us while the first tiles
            # land; a burst of throwaway matmuls in that window flips the
            # clock gate to 8/8 before the first real matmul issues.
            warm_state = {}

            def warm_mm(n):
                if "tile" not in warm_state:
                    wt = xpool.tile([128, 512], mybir.dt.float32, tag="warm",
                                    name="warm")
                    nc.vector.memset(wt[:], 0.0)
                    warm_state["tile"] = wt
                    warm_state["ps"] = pspool.tile(
                        [128, 512], mybir.dt.float32, tag="pswarm",
                        name="pswarm")
                wt, psw = warm_state["tile"], warm_state["ps"]
                for _ in range(n):
                    nc.tensor.matmul(psw[:],
                                     wt[:, :128].bitcast(DT),
                                     wt[:].bitcast(DT),
                                     start=True, stop=True)

            if WARMUP_MMS:
                warm_mm(WARMUP_MMS)

            # Startup choreography. The scalar queue takes ~4us to wake up,
            # so poke it with an 8-byte dummy DMA first; the gpsimd SWDGE
            # queue carries the two latest-needed chunks as a third stream.
            qwarm = wstage.tile([128, 8], mybir.dt.bfloat16, tag="qwarm",
                                name="qwarm")
            nc.scalar.dma_start(qwarm[:, :4], w.ap()[:, :4])
            wk = [None] * KT
            wk[0] = w_chunk(0, nc.sync)
            # first x tile, split so matmuls can start before the whole
            # 768KB tile has landed: k=0..2 first, then k=3..11
            X0A = 3
            x0a = xpool.tile([128, X0A * 128], DT, tag="x0a", name="x0a")
            nc.sync.dma_start(x0a[:], xt.ap()[0, :, :X0A * 128])
            x0b = xpool.tile([128, (KT - X0A) * 128], DT, tag="x0b",
                             name="x0b")
            nc.sync.dma_start(x0b[:], xt.ap()[0, :, X0A * 128:])
            wk[1] = w_chunk(1, nc.scalar)
            wk[2] = w_chunk(2, nc.sync)
            wk[3] = w_chunk(3, nc.scalar)
            wk[4] = w_chunk(4, nc.sync)
            wk[5] = w_chunk(5, nc.scalar)
            wk[6] = w_chunk(6, nc.sync)
            wk[7] = w_chunk(7, nc.scalar)
            wk[9] = w_chunk(9, nc.gpsimd)
            wk[11] = w_chunk(11, nc.gpsimd)
            x1 = xpool.tile([128, KT * 128], DT, tag="xbt", name="xb1")
            nc.sync.dma_start(x1[:], xt.ap()[1])
            wk[8] = w_chunk(8, nc.sync)
            wk[10] = w_chunk(10, nc.scalar)

            def load_x(bt):
                xbt = xpool.tile([128, KT * 128], DT, tag="xbt", name=f"xb{bt}")
                nc.sync.dma_start(xbt[:], xt.ap()[bt])
                return xbt

            xbt = None
            for bt in range(NBT):
                ps = [
                    pspool.tile([128, 512], mybir.dt.float32, tag=f"ps{g}",
                                name=f"ps{bt}_{g}")
                    for g in range(NG)
                ]
                for k in range(KT):
                    if bt == 0:
                        lhsT = (x0a[:, k * 128:(k + 1) * 128] if k < X0A
                                else x0b[:, (k - X0A) * 128:(k - X0A + 1) * 128])
                    else:
                        lhsT = xbt[:, k * 128:(k + 1) * 128]
                    for g in range(NG):
                        nc.tensor.matmul(
                            ps[g][:],
                            lhsT,
                            wk[k][:, g * 512:(g + 1) * 512].bitcast(DT),
                            start=(k == 0),
                            stop=(k == KT - 1),
                        )
                    # bridge the startup DMA race with no-dep filler matmuls
                    # so weight waits never idle the PE past the HAM window
                    if bt == 0 and 2 <= k < KT - 1:
                        warm_mm(2)
                    elif bt == 1 and k % 2 == 0:
                        warm_mm(1)
                if bt == 0:
                    nxt = x1
                elif bt + 1 < NBT:
                    nxt = load_x(bt + 1)
                ob = opool.tile([128, NG * 512], mybir.dt.float32, tag="ob",
                                name=f"ob{bt}")
                for g in range(NG):
                    nc.vector.tensor_copy(ob[:, g * 512:(g + 1) * 512],
                                          ps[g][:])
                nc.scalar.dma_start(
                    out_l.ap()[bt * 128:(bt + 1) * 128, :], ob[:])
                if bt + 1 < NBT:
                    xbt = nxt

    nc.compile()
    return nc


def _get_nc(dt_tag):
    if dt_tag not in _CACHE:
        _CACHE[dt_tag] = _build(dt_tag)
    return _CACHE[dt_tag]


def _np_dt(dt_tag):
    return ml_dtypes.bfloat16 if dt_tag == "bf16" else np.float32


def _pack_weight(weight, pair_orbit, dt_tag):
    # W_i[(j,c), o] = weight[o, c, pair_orbit[i, j]]
    kern = weight[:, :, np.asarray(pair_orbit)]          # (o, c, i, j)
    wfull = kern.transpose(2, 3, 1, 0).reshape(P, JC, C_OUT)   # (i, jc, o)
    # Wmov[k, g, kc, di*64+o] = wfull[g*8+di, k*128+kc, o]
    wmov = (
        wfull.reshape(NG, 8, KT, 128, C_OUT)
        .transpose(2, 0, 3, 1, 4)
        .reshape(KT * NG, 128, 512)
    )
    wsb = np.ascontiguousarray(
        wmov.transpose(1, 0, 2).reshape(128, KT * NG * 512), dtype=np.float32
    )
    np_dtw = (ml_dtypes.bfloat16
              if (W_BF16_WIRE or dt_tag == "bf16") else np.float32)
    return wsb.astype(np_dtw)


def _shard_x(x, dt_tag):
    # tile[bt, kc, k, b] = x[c*BL + bt*128 + b, k*128 + kc]
    x2 = x.reshape(B, JC).astype(_np_dt(dt_tag))
    out = []
    for c in range(N_CORES):
        xc = x2[c * BL:(c + 1) * BL].reshape(NBT, 128, KT, 128)
        out.append(
            np.ascontiguousarray(xc.transpose(0, 3, 2, 1))
            .reshape(NBT, 128, KT * 128)
        )
    return out


def kernel(x, weight, bias, pair_orbit):
    x = np.asarray(x, dtype=np.float32)
    weight = np.asarray(weight, dtype=np.float32)
    bias = np.asarray(bias, dtype=np.float32)

    dt_tag = COMPUTE_DTYPE
    nc = _get_nc(dt_tag)

    wsb = _pack_weight(weight, pair_orbit, dt_tag)
    xts = _shard_x(x, dt_tag)
    in_maps = [{"xt": xts[c], "w": wsb} for c in range(N_CORES)]

    res = run_bass_kernel_spmd(nc, in_maps, core_ids=list(range(N_CORES)))

    out = np.concatenate(
        [res.results[c]["out_l"] for c in range(N_CORES)], axis=0
    ).reshape(B, P, C_OUT)
    if bias.any():
        out = out + bias
    return out


# revision 33
# speedup vs baseline: 1.2014x; 1.2014x over previous
"""Trainium2 Bass kernel for a group-conv / orbit-shared message-passing layer.

Math: out[b, i, o] = sum_{j,c} weight[o, c, pair_orbit[i, j]] * x[b, j, c] + bias[o]

Strategy (pure data parallel over 8 NeuronCores):
  * Host gathers the orbit-shared weight into per-output-position matrices
    W_i[(j,c), o] (24 matrices of 1536x64), regrouped as moving operands
    Wmov[k, g][kc, (di,o)] of [128, 512] covering 8 output positions each.
  * Host transposes x to x^T[(j,c), b] so the contraction dim (j,c)=1536 sits
    on SBUF partitions; each core takes B/8 = 4096 batch columns.
  * Per 128-batch tile: stationary = x^T k-tile [kc=128, b=128] (one weight
    load per 3 matmuls with walrus ldw-opt dedupe), moving = Wmov[k, g]
    [kc=128, 512]; 12 k-tiles accumulate into 3 PSUM banks:
        psum_g[b, (di,o)] += xT[kc, b].T @ Wmov[k,g][kc, (di,o)]
    The PSUM free axis (di,o) is already the natural out[b, i, o] layout, so
    stores go straight to a (4096, 24*64) DRAM tensor. No host-side output
    transpose.
  * DMA queues are split by stream: x loads on sync, weight chunks on scalar,
    output stores on gpsimd -- so the big weight burst at startup does not
    queue behind x tiles and the output never backs up the input stream.
"""

import sys

for _p in ("/opt/trn_rl_repo",):
    if _p not in sys.path:
        sys.path.insert(0, _p)

import numpy as np
import ml_dtypes

import concourse.bacc as bacc
import concourse.mybir as mybir
from concourse import tile
from concourse.bass_utils import run_bass_kernel_spmd

B, P, C_IN, C_OUT, N_ORB = 32768, 24, 64, 64, 24
N_CORES = 8
BL = B // N_CORES            # 4096 batch per core
JC = P * C_IN                # 1536 contraction size
KT = JC // 128               # 12 K-tiles
NG = 3                       # output groups of 8 positions (8*64 = 512 free)
NBT = BL // 128              # 32 batch tiles per core

# "bf16" | "f32r" | "f32"
COMPUTE_DTYPE = "f32r"
# Ship weights over the wire as bf16 (half the startup DMA) and cast them to
# the compute dtype on-device with the Vector engine, which is idle during
# startup. Mixed-dtype matmuls are rejected by walrus, so the cast is needed.
W_BF16_WIRE = True
# Let walrus dedupe back-to-back LDWEIGHTS of the same stationary operand.
# Measured: dedupe is a net loss here (~+6ns/MM steady-state: the per-k-tile
# LDWEIGHTS->MATMUL serialization outweighs the removed instructions).
LDW_OPT = False
# Dummy matmuls issued while the first DMAs are in flight, so the HAM clock
# gate reaches 8/8 before the first real matmul.
WARMUP_MMS = 10

_CACHE = {}


def _patch_ldw_opt():
    import concourse.bass_utils as bu

    orig = bu.run_command
    if getattr(orig, "_ldw_patched", False):
        return

    def wrapper(argv, **kwargs):
        if LDW_OPT and "--enable-ldw-opt=false" in argv:
            argv = ["--enable-ldw-opt=true" if a == "--enable-ldw-opt=false" else a
                    for a in argv]
        return orig(argv, **kwargs)

    wrapper._ldw_patched = True
    bu.run_command = wrapper


def _dt(dt_tag):
    if dt_tag == "bf16":
        return mybir.dt.bfloat16
    if dt_tag == "f32r":
        return mybir.dt.float32r
    return mybir.dt.float32


def _build(dt_tag):
    _patch_ldw_opt()
    DT = _dt(dt_tag)
    wire_bf16 = W_BF16_WIRE and dt_tag != "bf16"
    DTW = mybir.dt.bfloat16 if wire_bf16 else DT

    nc = bacc.Bacc(None, target_bir_lowering=False, debug=False)
    # x pre-packed on host so each batch tile is one contiguous [128, 1536]
    # block (per-partition 6KB runs -> large DMA descriptors, not 512B shreds)
    xt = nc.dram_tensor("xt", [NBT, 128, KT * 128], DT, kind="ExternalInput")
    w = nc.dram_tensor("w", [128, KT * NG * 512], DTW, kind="ExternalInput")
    # row-major output: batch-tile rows are contiguous 768KB stores
    out_l = nc.dram_tensor("out_l", [BL, P * C_OUT], mybir.dt.float32,
                           kind="ExternalOutput")

    with tile.TileContext(nc) as tc:
        with (
            tc.tile_pool(name="wpool", bufs=1) as wpool,
            tc.tile_pool(name="wsta", bufs=1) as wstage,
            tc.tile_pool(name="xpool", bufs=3) as xpool,
            tc.tile_pool(name="opool", bufs=4) as opool,
            tc.tile_pool(name="pspool", bufs=2, space="PSUM") as pspool,
        ):
            def _vcast(out, in_):
                nc.vector.tensor_copy(out, in_)

            def _scast(out, in_):
                nc.scalar.copy(out, in_)

            cast_engs = [_vcast, _scast, _vcast]

            def w_chunk(k, eng):
                sl = slice(k * NG * 512, (k + 1) * NG * 512)
                if not wire_bf16:
                    wc = wpool.tile([128, NG * 512], DT, tag=f"w{k}",
                                    name=f"wc{k}")
                    eng.dma_start(wc[:], w.ap()[:, sl])
                    return wc
                # unique staging tile per chunk: chunk DMAs never wait on a
                # cast to release a reused buffer
                ws = wstage.tile([128, NG * 512], mybir.dt.bfloat16,
                                 tag=f"ws{k % 6}", name=f"ws{k}")
                eng.dma_start(ws[:], w.ap()[:, sl])
                wc = wpool.tile([128, NG * 512], DT, tag=f"w{k}",
                                name=f"wc{k}")
                # per-group casts spread over the idle compute engines
                for g in range(NG):
                    cast_engs[g](
                        wc[:, g * 512:(g + 1) * 512],
                        ws[:, g * 512:(g + 1) * 512],
                    )
                return wc

            # HAM warmup: the PE sits idle for ~5us while the first tiles
            # land; a burst of throwaway matmuls in that window flips the
            # clock gate to 8/8 before the first real matmul issues.
            warm_state = {}

            def warm_mm(n):
                if "tile" not in warm_state:
                    wt = xpool.tile([128, 512], mybir.dt.float32, tag="warm",
                                    name="warm")
                    nc.vector.memset(wt[:], 0.0)
                    warm_state["tile"] = wt
                    warm_state["ps"] = pspool.tile(
                        [128, 512], mybir.dt.float32, tag="pswarm",
                        name="pswarm")
                wt, psw = warm_state["tile"], warm_state["ps"]
                for _ in range(n):
                    nc.tensor.matmul(psw[:],
                                     wt[:, :128].bitcast(DT),
                                     wt[:].bitcast(DT),
                                     start=True, stop=True)

            if WARMUP_MMS:
                warm_mm(WARMUP_MMS)

            # Startup choreography. The scalar queue takes ~4us to wake up,
            # so poke it with an 8-byte dummy DMA first; the gpsimd SWDGE
            # queue carries the two latest-needed chunks as a third stream.
            qwarm = wstage.tile([128, 8], mybir.dt.bfloat16, tag="qwarm",
                                name="qwarm")
            nc.scalar.dma_start(qwarm[:, :4], w.ap()[:, :4])
            wk = [None] * KT
            wk[0] = w_chunk(0, nc.sync)
            # first x tile, split so matmuls can start before the whole
            # 768KB tile has landed: k=0..2 first, then k=3..11
            X0A = 3
            x0a = xpool.tile([128, X0A * 128], DT, tag="x0a", name="x0a")
            nc.sync.dma_start(x0a[:], xt.ap()[0, :, :X0A * 128])
            x0b = xpool.tile([128, (KT - X0A) * 128], DT, tag="x0b",
                             name="x0b")
            nc.sync.dma_start(x0b[:], xt.ap()[0, :, X0A * 128:])
            wk[1] = w_chunk(1, nc.scalar)
            wk[2] = w_chunk(2, nc.sync)
            wk[3] = w_chunk(3, nc.scalar)
            wk[4] = w_chunk(4, nc.sync)
            wk[5] = w_chunk(5, nc.scalar)
            wk[6] = w_chunk(6, nc.sync)
            wk[7] = w_chunk(7, nc.scalar)
            x1 = xpool.tile([128, KT * 128], DT, tag="xbt", name="xb1")
            nc.sync.dma_start(x1[:], xt.ap()[1])
            wk[8] = w_chunk(8, nc.sync)
            wk[9] = w_chunk(9, nc.scalar)
            wk[10] = w_chunk(10, nc.sync)
            wk[11] = w_chunk(11, nc.scalar)

            def load_x(bt):
                xbt = xpool.tile([128, KT * 128], DT, tag="xbt", name=f"xb{bt}")
                nc.sync.dma_start(xbt[:], xt.ap()[bt])
                return xbt

            xbt = None
            for bt in range(NBT):
                ps = [
                    pspool.tile([128, 512], mybir.dt.float32, tag=f"ps{g}",
                                name=f"ps{bt}_{g}")
                    for g in range(NG)
                ]
                for k in range(KT):
                    if bt == 0:
                        lhsT = (x0a[:, k * 128:(k + 1) * 128] if k < X0A
                                else x0b[:, (k - X0A) * 128:(k - X0A + 1) * 128])
                    else:
                        lhsT = xbt[:, k * 128:(k + 1) * 128]
                    for g in range(NG):
                        nc.tensor.matmul(
                            ps[g][:],
                            lhsT,
                            wk[k][:, g * 512:(g + 1) * 512].bitcast(DT),
                            start=(k == 0),
                            stop=(k == KT - 1),
                        )
                    # bridge the startup DMA race with no-dep filler matmuls
                    # so weight waits never idle the PE past the HAM window
                    if bt == 0 and 2 <= k < KT - 1:
                        warm_mm(2)
                    elif bt == 1 and k % 2 == 0:
                        warm_mm(1)
                if bt == 0:
                    nxt = x1
                elif bt + 1 < NBT:
                    nxt = load_x(bt + 1)
                ob = opool.tile([128, NG * 512], mybir.dt.float32, tag="ob",
                                name=f"ob{bt}")
                for g in range(NG):
                    nc.vector.tensor_copy(ob[:, g * 512:(g + 1) * 512],
                                          ps[g][:])
                nc.scalar.dma_start(
                    out_l.ap()[bt * 128:(bt + 1) * 128, :], ob[:])
                if bt + 1 < NBT:
                    xbt = nxt

    nc.compile()
    return nc


def _get_nc(dt_tag):
    if dt_tag not in _CACHE:
        _CACHE[dt_tag] = _build(dt_tag)
    return _CACHE[dt_tag]


def _np_dt(dt_tag):
    return ml_dtypes.bfloat16 if dt_tag == "bf16" else np.float32


def _pack_weight(weight, pair_orbit, dt_tag):
    # W_i[(j,c), o] = weight[o, c, pair_orbit[i, j]]
    kern = weight[:, :, np.asarray(pair_orbit)]          # (o, c, i, j)
    wfull = kern.transpose(2, 3, 1, 0).reshape(P, JC, C_OUT)   # (i, jc, o)
    # Wmov[k, g, kc, di*64+o] = wfull[g*8+di, k*128+kc, o]
    wmov = (
        wfull.reshape(NG, 8, KT, 128, C_OUT)
        .transpose(2, 0, 3, 1, 4)
        .reshape(KT * NG, 128, 512)
    )
    wsb = np.ascontiguousarray(
        wmov.transpose(1, 0, 2).reshape(128, KT * NG * 512), dtype=np.float32
    )
    np_dtw = (ml_dtypes.bfloat16
              if (W_BF16_WIRE or dt_tag == "bf16") else np.float32)
    return wsb.astype(np_dtw)


def _shard_x(x, dt_tag):
    # tile[bt, kc, k, b] = x[c*BL + bt*128 + b, k*128 + kc]
    x2 = x.reshape(B, JC).astype(_np_dt(dt_tag))
    out = []
    for c in range(N_CORES):
        xc = x2[c * BL:(c + 1) * BL].reshape(NBT, 128, KT, 128)
        out.append(
            np.ascontiguousarray(xc.transpose(0, 3, 2, 1))
            .reshape(NBT, 128, KT * 128)
        )
    return out


def kernel(x, weight, bias, pair_orbit):
    x = np.asarray(x, dtype=np.float32)
    weight = np.asarray(weight, dtype=np.float32)
    bias = np.asarray(bias, dtype=np.float32)

    dt_tag = COMPUTE_DTYPE
    nc = _get_nc(dt_tag)

    wsb = _pack_weight(weight, pair_orbit, dt_tag)
    xts = _shard_x(x, dt_tag)
    in_maps = [{"xt": xts[c], "w": wsb} for c in range(N_CORES)]

    res = run_bass_kernel_spmd(nc, in_maps, core_ids=list(range(N_CORES)))

    out = np.concatenate(
        [res.results[c]["out_l"] for c in range(N_CORES)], axis=0
    ).reshape(B, P, C_OUT)
    if bias.any():
        out = out + bias
    return out
